# revision 36
# baseline (speedup 1.0000x reference)
"""Trainium2 Bass kernel for nn_AttentionLayer (B=8, S=1024, D=1024, H=16, HD=64).

Strategy: pure data parallelism — one batch element per NeuronCore (8 cores).
Weights are replicated (pre-transposed on host so the contraction dim lands on
SBUF partitions); x is sharded on batch and pre-transposed per shard.

Per-core compute layout (all transposes eliminated by construction):
  Qt/Kt [dout, s]  = W^T-stacked @ x^T         (d on partitions)
  Vx    [s, dout]  natural, 65-strided per head with a ones column; padded
                    keys' rows are zeroed (this IS the pad mask: they then
                    contribute 0 to both attention output and denominator)
  scoresT[k, q]    = Kt_h^T @ Qt_h             (k on partitions, q free);
                    chunks start at the causal diagonal (qs = 128*c), so
                    only the trapezoid is computed. Causal masking of the
                    128-col diagonal block = identity-matmul accumulating a
                    -1e9 triangle mask into the scores PSUM.
  expT   [k, q]    = exp(scoresT / 8)          (ACT, PSUM->SBUF, bf16 out)
  avT -> out[q, d] via lhsT=[V_h | 1]: ones column also produces the softmax
                    denominator as psum row 64; accumulated per 512-wide
                    q-chunk (4 PSUM banks per pair) with per-key-block
                    column trimming to the causal-active range.
  normalization    fully on-chip: DVE reciprocal reads the denominator row
                    straight from PSUM, GpSimd partition_broadcast fans it
                    out to the 64 head rows, and the aoT *= recip multiply
                    is software-pipelined one head pair behind so the
                    Vector queue never blocks the PE's PSUM reuse.
  out [s, dout]    = attn_outT^T @ Wo^T + bo   (bias via K=1 matmul)

Matmul dtype: bf16 (f32r would be full fp32 precision at the same PE rate,
but hangs TRN2 hardware - observed empirically). End-to-end rel err vs the
fp32 reference is ~4e-3.
"""

import os
import sys
import types

import numpy as np

B, S, D, H, HD = 8, 1024, 1024, 16, 64
NT = D // 128          # 8 partition tiles
PAD_ID = 1.0
NEG = -1e9
SCALE = 1.0 / 8.0      # 1/sqrt(HD)

MM_MODE = os.environ.get("KERNEL_MM_MODE", "bf16")

_CACHE = {}
LAST_RESULT = None
LAST_EXEC_NS = None


def _install_trace_hook():
    """Provide antenv.axon_hooks (missing in this image) so trace=True works."""
    try:
        import antenv
        if "antenv.axon_hooks" in sys.modules:
            return True
        m = types.ModuleType("antenv.axon_hooks")
        _hook = [None]
        m.set_axon_ntff_profile_hook = lambda h: _hook.__setitem__(0, h)
        m.get_axon_ntff_profile_hook = lambda: _hook[0]
        sys.modules["antenv.axon_hooks"] = m
        antenv.axon_hooks = m
        from trn_agent_boot.trn_boot import _ntff_profile_via_ctypes
        hook = _ntff_profile_via_ctypes("/opt/axon/libaxon_pjrt.so")
        if hook is None:
            return False
        m.set_axon_ntff_profile_hook(hook)
        return True
    except Exception:
        return False


def _build_graph():
    import concourse.bass as bass
    import concourse.mybir as mybir
    import concourse.tile as tile
    from concourse import bacc
    from concourse import library_config

    F32 = mybir.dt.float32
    MMD = {"bf16": mybir.dt.bfloat16, "f32r": mybir.dt.float32r,
           "f32": mybir.dt.float32}[MM_MODE]
    AluOp = mybir.AluOpType
    Act = mybir.ActivationFunctionType

    nc = bacc.Bacc(target_bir_lowering=False)

    xT = nc.declare_dram_parameter("xT", [D, S], MMD, isOutput=False)
    WqT = nc.declare_dram_parameter("WqT", [D, D], MMD, isOutput=False)
    WkT = nc.declare_dram_parameter("WkT", [D, D], MMD, isOutput=False)
    WvT = nc.declare_dram_parameter("WvT", [D, D], MMD, isOutput=False)
    WoT = nc.declare_dram_parameter("WoT", [D, D], MMD, isOutput=False)
    bv = nc.declare_dram_parameter("bv", [D], MMD, isOutput=False)
    bo = nc.declare_dram_parameter("bo", [D], MMD, isOutput=False)
    ones_p = nc.declare_dram_parameter("ones", [S], MMD, isOutput=False)
    # smalls: [128, 24] f32 = ids_r | bq_r | bk_r (each [128, 8], host-packed)
    smalls = nc.declare_dram_parameter("smalls", [128, 3 * NT], F32, isOutput=False)
    # causal triangle mask for the 128-wide diagonal block
    masks_p = nc.declare_dram_parameter("masks", [128, 128], MMD, isOutput=False)
    ident_p = nc.declare_dram_parameter("ident", [128, 128], MMD, isOutput=False)
    out_e = nc.declare_dram_parameter("out", [S, D], F32, isOutput=True)

    with tile.TileContext(nc) as tc:
        # partition_broadcast lives in the `attn` gpsimd library
        nc.gpsimd.load_library(library_config.attn)
        with tc.tile_pool(name="const", bufs=1) as cp, \
             tc.tile_pool(name="qkv", bufs=1) as qp:

            # ---- constants (scalar-triggered DMAs: keep the sync queue
            # free for the x/W streams the first matmuls wait on) ----
            sm = cp.tile([128, 3 * NT], F32, tag="sm", name="sm")
            nc.scalar.dma_start(out=sm[:], in_=smalls[:])
            pad01 = cp.tile([128, NT], F32, tag="pad01", name="pad01")
            nc.vector.tensor_scalar(out=pad01[:], in0=sm[:, 0:NT],
                                    scalar1=PAD_ID, scalar2=None,
                                    op0=AluOp.not_equal)
            bq_col = sm[:, NT:2 * NT]
            bk_col = sm[:, 2 * NT:3 * NT]
            bv_row = cp.tile([1, D], MMD, tag="bvr", name="bv_row")
            nc.scalar.dma_start(out=bv_row[:], in_=bv[None, :])
            bo_row = cp.tile([1, D], MMD, tag="bor", name="bo_row")
            nc.scalar.dma_start(out=bo_row[:], in_=bo[None, :])
            ones_row = cp.tile([1, S], MMD, tag="ones", name="ones_row")
            nc.scalar.dma_start(out=ones_row[:], in_=ones_p[None, :])
            masks_sb = cp.tile([128, 128], MMD, tag="masks", name="masks_sb")
            nc.scalar.dma_start(out=masks_sb[:], in_=masks_p[:])
            ident = cp.tile([128, 128], MMD, tag="ident", name="ident")
            nc.scalar.dma_start(out=ident[:], in_=ident_p[:])

            # ---- persistent per-core tensors ----
            Qt = [qp.tile([128, S], MMD, tag=f"qt{t}", name=f"qt{t}")
                  for t in range(NT)]
            Kt = [qp.tile([128, S], MMD, tag=f"kt{t}", name=f"kt{t}")
                  for t in range(NT)]
            Vx = [qp.tile([128, H * (HD + 1)], MMD, tag=f"vx{t}", name=f"vx{t}")
                  for t in range(NT)]

            # ============ Phase B: projections ============
            # V first, then Q/K interleaved per output tile, so attention
            # head-pair t unblocks as soon as Qt[t]/Kt[t] land (instead of
            # after the whole K projection) and the PE never drains across
            # the phase boundary.
            with tc.tile_pool(name="xw", bufs=1) as xp, \
                 tc.tile_pool(name="wst", bufs=8) as wp, \
                 tc.tile_pool(name="wqk", bufs=16) as wqkp:

                # interleave x / Wv column-half DMAs so the c-outermost V
                # projection starts after the first pair lands instead of
                # after the whole 4MB
                xT_sb = [xp.tile([128, S], MMD, tag=f"x{c}", name=f"x{c}")
                         for c in range(NT)]
                w_sb = [wp.tile([128, D], MMD, tag="w", name="w_t")
                        for c in range(NT)]
                for c in range(NT):
                    nc.sync.dma_start(out=xT_sb[c][:], in_=xT[c * 128:(c + 1) * 128, :])
                    nc.scalar.dma_start(out=w_sb[c][:, 0:512],
                                        in_=WvT[c * 128:(c + 1) * 128, 0:512])
                for c in range(NT):
                    nc.gpsimd.dma_start(out=w_sb[c][:, 512:D],
                                        in_=WvT[c * 128:(c + 1) * 128, 512:D])

                def stream_w(w_ext, pool, eng):
                    tiles = []
                    for c in range(NT):
                        t = pool.tile([128, D], MMD, tag="w", name="w_t")
                        eng.dma_start(out=t[:], in_=w_ext[c * 128:(c + 1) * 128, :])
                        tiles.append(t)
                    return tiles

                # parallel hardware DMA queues: Q weights via the scalar
                # queue, K weights via the gpsimd queue
                wq_sb = stream_w(WqT, wqkp, nc.scalar)
                wk_sb = stream_w(WkT, wqkp, nc.gpsimd)

                # ones column FIRST per head: the softmax denominator then
                # lands on PSUM partition 0, where the gpsimd
                # partition_broadcast contract wants its source
                vdsts = []
                for m in range(NT):
                    vdst = Vx[m][:].rearrange("p (h e) -> p h e", e=HD + 1)
                    nc.vector.memset(vdst[:, :, 0:1], 1.0)
                    vdsts.append(vdst)
                with tc.tile_pool(name="psv", bufs=1, space="PSUM") as ppv:
                    psV = {}
                    for n in range(2):
                        for c in range(NT):
                            for m in range(NT):
                                if c == 0:
                                    psV[m] = ppv.tile([128, 512], F32,
                                                      tag=f"pv{m}",
                                                      name=f"pv{m}")
                                nc.tensor.matmul(
                                    psV[m][:],
                                    xT_sb[c][:, m * 128:(m + 1) * 128],
                                    w_sb[c][:, n * 512:(n + 1) * 512],
                                    start=(c == 0), stop=False)
                        for m in range(NT):
                            nc.tensor.matmul(psV[m][:], ones_row[:, :128],
                                             bv_row[:, n * 512:(n + 1) * 512],
                                             start=False, stop=True)
                            nc.vector.tensor_copy(
                                out=vdsts[m][:, n * 8:(n + 1) * 8, 1:HD + 1],
                                in_=psV[m][:].rearrange("p (h e) -> p h e",
                                                        e=HD))
                for m in range(NT):
                    # pad mask: zero whole rows (keys) where ids == PAD,
                    # including the ones column -> denominator excludes them
                    nc.vector.tensor_scalar(
                        out=Vx[m][:], in0=Vx[m][:],
                        scalar1=pad01[:, m:m + 1], scalar2=None,
                        op0=AluOp.mult)

                with tc.tile_pool(name="psp", bufs=4,
                                  space="PSUM") as pp:
                    for m in range(NT):
                        for w_sb2, dst, bias_col in ((wq_sb, Qt, bq_col),
                                                     (wk_sb, Kt, bk_col)):
                            for n in range(2):
                                ps = pp.tile([128, 512], F32, tag="pp",
                                             name="ps")
                                for c in range(NT):
                                    nc.tensor.matmul(
                                        ps[:],
                                        w_sb2[c][:, m * 128:(m + 1) * 128],
                                        xT_sb[c][:, n * 512:(n + 1) * 512],
                                        start=(c == 0), stop=(c == NT - 1))
                                nc.vector.tensor_scalar(
                                    out=dst[m][:, n * 512:(n + 1) * 512],
                                    in0=ps[:],
                                    scalar1=bias_col[:, m:m + 1], scalar2=None,
                                    op0=AluOp.add)

            # ============ Phase C: attention (head pairs) ============
            with tc.tile_pool(name="aot", bufs=1) as ap_pool, \
                 tc.tile_pool(name="wo", bufs=8) as wop:
              aoT = [ap_pool.tile([128, S], MMD, tag=f"ao{t}", name=f"ao{t}")
                     for t in range(NT)]
              with tc.tile_pool(name="expp", bufs=6) as ep, \
                   tc.tile_pool(name="avst", bufs=3) as avs, \
                   tc.tile_pool(name="rrp", bufs=2) as rrp, \
                   tc.tile_pool(name="rbp", bufs=2) as rbp, \
                   tc.tile_pool(name="pssc", bufs=2, space="PSUM") as psc, \
                   tc.tile_pool(name="psav", bufs=1, space="PSUM") as pav:

                # prefetch Wo during attention
                wo_sb = []
                for c in range(NT):
                    w_t = wop.tile([128, D], MMD, tag="wo", name="wo_t")
                    nc.sync.dma_start(out=w_t[:], in_=WoT[c * 128:(c + 1) * 128, :])
                    wo_sb.append(w_t)

                pend = None      # (pair idx, recb tile) awaiting normalization
                for t in range(NT):         # head pair (2t, 2t+1)
                    av_ps = {(par, g): pav.tile([HD + 1, 512], F32,
                                                tag=f"av{par}{g}",
                                                name=f"av{par}{g}")
                             for par in range(2) for g in range(2)}
                    # AV matmuls are emitted one c-iteration behind the
                    # scores, so on the in-order PE queue the next chunk's
                    # score matmuls fill the exp-ACT latency and the PE
                    # never idles (idle resets the 1.2->2.4GHz p-state ramp)
                    def emit_av(c, exs):
                        qs = 128 * c
                        for par in range(2):
                            for g in range(2):
                                if c <= 4 * g + 3:
                                    cs = max(0, qs - 512 * g)
                                    nc.tensor.matmul(
                                        av_ps[(par, g)][:, cs:512],
                                        Vx[c][:, (2 * t + par) * (HD + 1):
                                              (2 * t + par + 1) * (HD + 1)],
                                        exs[par][:, 512 * g + cs:
                                                 512 * (g + 1)],
                                        start=(c == 0),
                                        stop=(c == min(4 * g + 3, NT - 1)))

                    prev = None
                    for c in range(NT):
                        qs = 128 * c        # chunk starts at the diagonal
                        exs = {}
                        for par in range(2):
                            base = par * 64
                            sc = psc.tile([128, 1024], F32, tag="sc", name="sc")
                            if qs < 512:
                                nc.tensor.matmul(
                                    sc[:, qs:512],
                                    Kt[t][base:base + 64, c * 128:(c + 1) * 128],
                                    Qt[t][base:base + 64, qs:512],
                                    start=True, stop=False)
                                nc.tensor.matmul(
                                    sc[:, 512:S],
                                    Kt[t][base:base + 64, c * 128:(c + 1) * 128],
                                    Qt[t][base:base + 64, 512:S],
                                    start=True, stop=True)
                            else:
                                nc.tensor.matmul(
                                    sc[:, qs:S],
                                    Kt[t][base:base + 64, c * 128:(c + 1) * 128],
                                    Qt[t][base:base + 64, qs:S],
                                    start=True, stop=False)
                            # causal triangle on the diagonal block via
                            # identity-matmul accumulation
                            nc.tensor.matmul(
                                sc[:, qs:qs + 128], ident[:], masks_sb[:],
                                start=False, stop=True)
                            ex = ep.tile([128, 1024], MMD, tag="ex",
                                         name="ex")
                            nc.scalar.activation(out=ex[:, qs:S],
                                                 in_=sc[:, qs:S],
                                                 func=Act.Exp, scale=SCALE)
                            exs[par] = ex
                        if prev is not None:
                            emit_av(*prev)
                        prev = (c, exs)
                    emit_av(*prev)
                    # softmax denominators: fp32 approx-reciprocal (fast
                    # custom DVE op) straight off the PSUM ones-row, fanned
                    # out by gpsimd partition_broadcast, with the aoT
                    # multiply software-pipelined one pair behind — fully
                    # on-chip, nothing ever blocks a DMA queue
                    def finish_pair(pt, precb):
                        for par in range(2):
                            nc.vector.tensor_mul(
                                aoT[pt][par * 64:(par + 1) * 64, :],
                                aoT[pt][par * 64:(par + 1) * 64, :],
                                precb[par][par * 64:(par + 1) * 64, :])

                    rrow = rrp.tile([1, 2 * S], F32, tag="rr", name="rrow")
                    for par in range(2):
                        for g in range(2):
                            nc.vector.reciprocal_approx_fast(
                                out=rrow[0:1, par * S + 512 * g:
                                         par * S + 512 * (g + 1)],
                                in_=av_ps[(par, g)][0:1, :])
                    for par in range(2):
                        st = avs.tile([HD + 1, S], MMD, tag="st", name="st")
                        for g in range(2):
                            nc.vector.tensor_copy(
                                out=st[:, 512 * g:512 * (g + 1)],
                                in_=av_ps[(par, g)][:])
                        nc.sync.dma_start(
                            out=aoT[t][par * 64:par * 64 + HD, :],
                            in_=st[1:HD + 1, :])
                    recb = [rbp.tile([128, S], F32, tag=f"recb{par}",
                                     name=f"recb{par}")
                            for par in range(2)]
                    for par in range(2):
                        nc.gpsimd.partition_broadcast(
                            recb[par][:], rrow[0:1, par * S:(par + 1) * S],
                            channels=128)
                    if pend is not None:
                        finish_pair(*pend)
                    pend = (t, recb)
                finish_pair(*pend)

              # ============ Phase E: output projection ============
              with tc.tile_pool(name="ost", bufs=3) as osp, \
                   tc.tile_pool(name="psf", bufs=4, space="PSUM") as pf:
                for m in range(NT):
                    for n in range(2):
                        ps = pf.tile([128, 512], F32, tag="pf", name="psf")
                        for c in range(NT):
                            nc.tensor.matmul(
                                ps[:],
                                aoT[c][:, m * 128:(m + 1) * 128],
                                wo_sb[c][:, n * 512:(n + 1) * 512],
                                start=(c == 0), stop=False)
                        nc.tensor.matmul(ps[:], ones_row[:, :128],
                                         bo_row[:, n * 512:(n + 1) * 512],
                                         start=False, stop=True)
                        ot = osp.tile([128, 512], F32, tag="ot", name="ot")
                        nc.scalar.copy(out=ot[:], in_=ps[:])
                        nc.sync.dma_start(
                            out=out_e[m * 128:(m + 1) * 128, n * 512:(n + 1) * 512],
                            in_=ot[:])
    nc.finalize()
    return nc


def _np_mm_dtype():
    if MM_MODE == "bf16":
        import ml_dtypes
        return ml_dtypes.bfloat16
    return np.float32


def _host_consts(mmdt):
    jj = np.arange(128)[None, :]
    pp = np.arange(128)[:, None]
    masks = np.where(jj < pp, NEG, 0.0).astype(mmdt)
    ident = np.eye(128, dtype=np.float32).astype(mmdt)
    return masks, ident


def build_in_maps(x, input_ids, Wq, bq, Wk, bk, Wv, bv, Wo, bo):
    x = np.asarray(x, dtype=np.float32)
    input_ids = np.asarray(input_ids)
    mmdt = _np_mm_dtype()
    masks, ident = _host_consts(mmdt)
    bq_r = np.ascontiguousarray(np.asarray(bq, np.float32).reshape(NT, 128).T)
    bk_r = np.ascontiguousarray(np.asarray(bk, np.float32).reshape(NT, 128).T)
    shared = {
        "WqT": np.ascontiguousarray(np.asarray(Wq, np.float32).T).astype(mmdt),
        "WkT": np.ascontiguousarray(np.asarray(Wk, np.float32).T).astype(mmdt),
        "WvT": np.ascontiguousarray(np.asarray(Wv, np.float32).T).astype(mmdt),
        "WoT": np.ascontiguousarray(np.asarray(Wo, np.float32).T).astype(mmdt),
        "bv": np.asarray(bv, np.float32).astype(mmdt),
        "bo": np.asarray(bo, np.float32).astype(mmdt),
        "ones": np.ones([S], mmdt),
        "masks": masks, "ident": ident,
    }
    in_maps = []
    for b in range(B):
        ids_r = input_ids[b].astype(np.float32).reshape(NT, 128).T
        m = dict(shared)
        m["xT"] = np.ascontiguousarray(x[b].T).astype(mmdt)
        m["smalls"] = np.ascontiguousarray(
            np.concatenate([ids_r, bq_r, bk_r], axis=1)).astype(np.float32)
        in_maps.append(m)
    return in_maps


def kernel(x, input_ids, Wq, bq, Wk, bk, Wv, bv, Wo, bo):
    global LAST_RESULT, LAST_EXEC_NS
    from concourse.bass_utils import run_bass_kernel_spmd

    if "nc" not in _CACHE:
        _CACHE["nc"] = _build_graph()
    nc = _CACHE["nc"]
    in_maps = build_in_maps(x, input_ids, Wq, bq, Wk, bk, Wv, bv, Wo, bo)

    trace = os.environ.get("KERNEL_TRACE", "0") == "1" and _install_trace_hook()
    res = run_bass_kernel_spmd(nc, in_maps, core_ids=list(range(B)), trace=trace)
    LAST_RESULT = res
    LAST_EXEC_NS = res.exec_time_ns
    return np.stack([res.results[b]["out"] for b in range(B)]).astype(np.float32)



# revision 37
# speedup vs baseline: 1.0384x; 1.0384x over previous
"""Trainium2 Bass kernel for nn_AttentionLayer (B=8, S=1024, D=1024, H=16, HD=64).

Strategy: pure data parallelism — one batch element per NeuronCore (8 cores).
Weights are replicated (pre-transposed on host so the contraction dim lands on
SBUF partitions); x is sharded on batch and pre-transposed per shard.

Per-core compute layout (all transposes eliminated by construction):
  Qt/Kt [dout, s]  = W^T-stacked @ x^T         (d on partitions)
  Vx    [s, dout]  natural, 65-strided per head with a ones column; padded
                    keys' rows are zeroed (this IS the pad mask: they then
                    contribute 0 to both attention output and denominator)
  scoresT[k, q]    = Kt_h^T @ Qt_h             (k on partitions, q free);
                    chunks start at the causal diagonal (qs = 128*c), so
                    only the trapezoid is computed. Causal masking of the
                    128-col diagonal block = identity-matmul accumulating a
                    -1e9 triangle mask into the scores PSUM.
  expT   [k, q]    = exp(scoresT / 8)          (ACT, PSUM->SBUF, bf16 out)
  avT -> out[q, d] via lhsT=[V_h | 1]: ones column also produces the softmax
                    denominator as psum row 64; accumulated per 512-wide
                    q-chunk (4 PSUM banks per pair) with per-key-block
                    column trimming to the causal-active range.
  normalization    fully on-chip: DVE reciprocal reads the denominator row
                    straight from PSUM, GpSimd partition_broadcast fans it
                    out to the 64 head rows, and the aoT *= recip multiply
                    is software-pipelined one head pair behind so the
                    Vector queue never blocks the PE's PSUM reuse.
  out [s, dout]    = attn_outT^T @ Wo^T + bo   (bias via K=1 matmul)

Matmul dtype: bf16 (f32r would be full fp32 precision at the same PE rate,
but hangs TRN2 hardware - observed empirically). End-to-end rel err vs the
fp32 reference is ~4e-3.
"""

import os
import sys
import types

import numpy as np

B, S, D, H, HD = 8, 1024, 1024, 16, 64
NT = D // 128          # 8 partition tiles
PAD_ID = 1.0
NEG = -1e9
SCALE = 1.0 / 8.0      # 1/sqrt(HD)

MM_MODE = os.environ.get("KERNEL_MM_MODE", "bf16")

_CACHE = {}
LAST_RESULT = None
LAST_EXEC_NS = None


def _install_trace_hook():
    """Provide antenv.axon_hooks (missing in this image) so trace=True works."""
    try:
        import antenv
        if "antenv.axon_hooks" in sys.modules:
            return True
        m = types.ModuleType("antenv.axon_hooks")
        _hook = [None]
        m.set_axon_ntff_profile_hook = lambda h: _hook.__setitem__(0, h)
        m.get_axon_ntff_profile_hook = lambda: _hook[0]
        sys.modules["antenv.axon_hooks"] = m
        antenv.axon_hooks = m
        from trn_agent_boot.trn_boot import _ntff_profile_via_ctypes
        hook = _ntff_profile_via_ctypes("/opt/axon/libaxon_pjrt.so")
        if hook is None:
            return False
        m.set_axon_ntff_profile_hook(hook)
        return True
    except Exception:
        return False


def _build_graph():
    import concourse.bass as bass
    import concourse.mybir as mybir
    import concourse.tile as tile
    from concourse import bacc
    from concourse import library_config

    F32 = mybir.dt.float32
    MMD = {"bf16": mybir.dt.bfloat16, "f32r": mybir.dt.float32r,
           "f32": mybir.dt.float32}[MM_MODE]
    AluOp = mybir.AluOpType
    Act = mybir.ActivationFunctionType

    nc = bacc.Bacc(target_bir_lowering=False)

    xT = nc.declare_dram_parameter("xT", [D, S], MMD, isOutput=False)
    WqT = nc.declare_dram_parameter("WqT", [D, D], MMD, isOutput=False)
    WkT = nc.declare_dram_parameter("WkT", [D, D], MMD, isOutput=False)
    WvT = nc.declare_dram_parameter("WvT", [D, D], MMD, isOutput=False)
    WoT = nc.declare_dram_parameter("WoT", [D, D], MMD, isOutput=False)
    bv = nc.declare_dram_parameter("bv", [D], MMD, isOutput=False)
    bo = nc.declare_dram_parameter("bo", [D], MMD, isOutput=False)
    ones_p = nc.declare_dram_parameter("ones", [S], MMD, isOutput=False)
    # smalls: [128, 24] f32 = ids_r | bq_r | bk_r (each [128, 8], host-packed)
    smalls = nc.declare_dram_parameter("smalls", [128, 3 * NT], F32, isOutput=False)
    # 0/1 causal triangle for the 128-wide diagonal block (applied to
    # exp output on the Vector engine, not via PE identity-matmul)
    masks_p = nc.declare_dram_parameter("masks", [128, 128], MMD, isOutput=False)
    out_e = nc.declare_dram_parameter("out", [S, D], F32, isOutput=True)

    with tile.TileContext(nc) as tc:
        # partition_broadcast lives in the `attn` gpsimd library
        nc.gpsimd.load_library(library_config.attn)
        with tc.tile_pool(name="const", bufs=1) as cp, \
             tc.tile_pool(name="qkv", bufs=1) as qp:

            # ---- constants (scalar-triggered DMAs: keep the sync queue
            # free for the x/W streams the first matmuls wait on) ----
            sm = cp.tile([128, 3 * NT], F32, tag="sm", name="sm")
            nc.scalar.dma_start(out=sm[:], in_=smalls[:])
            pad01 = cp.tile([128, NT], F32, tag="pad01", name="pad01")
            nc.vector.tensor_scalar(out=pad01[:], in0=sm[:, 0:NT],
                                    scalar1=PAD_ID, scalar2=None,
                                    op0=AluOp.not_equal)
            bq_col = sm[:, NT:2 * NT]
            bk_col = sm[:, 2 * NT:3 * NT]
            bv_row = cp.tile([1, D], MMD, tag="bvr", name="bv_row")
            nc.scalar.dma_start(out=bv_row[:], in_=bv[None, :])
            bo_row = cp.tile([1, D], MMD, tag="bor", name="bo_row")
            nc.scalar.dma_start(out=bo_row[:], in_=bo[None, :])
            ones_row = cp.tile([1, S], MMD, tag="ones", name="ones_row")
            nc.scalar.dma_start(out=ones_row[:], in_=ones_p[None, :])
            masks_sb = cp.tile([128, 128], MMD, tag="masks", name="masks_sb")
            nc.scalar.dma_start(out=masks_sb[:], in_=masks_p[:])

            # ---- persistent per-core tensors ----
            Qt = [qp.tile([128, S], MMD, tag=f"qt{t}", name=f"qt{t}")
                  for t in range(NT)]
            Kt = [qp.tile([128, S], MMD, tag=f"kt{t}", name=f"kt{t}")
                  for t in range(NT)]
            Vx = [qp.tile([128, H * (HD + 1)], MMD, tag=f"vx{t}", name=f"vx{t}")
                  for t in range(NT)]

            # ============ Phase B: projections ============
            # V first, then Q/K interleaved per output tile, so attention
            # head-pair t unblocks as soon as Qt[t]/Kt[t] land (instead of
            # after the whole K projection) and the PE never drains across
            # the phase boundary.
            with tc.tile_pool(name="xw", bufs=1) as xp, \
                 tc.tile_pool(name="wst", bufs=8) as wp, \
                 tc.tile_pool(name="wqk", bufs=16) as wqkp:

                # interleave x / Wv column-half DMAs so the c-outermost V
                # projection starts after the first pair lands instead of
                # after the whole 4MB
                xT_sb = [xp.tile([128, S], MMD, tag=f"x{c}", name=f"x{c}")
                         for c in range(NT)]
                w_sb = [wp.tile([128, D], MMD, tag="w", name="w_t")
                        for c in range(NT)]
                for c in range(NT):
                    nc.sync.dma_start(out=xT_sb[c][:], in_=xT[c * 128:(c + 1) * 128, :])
                    nc.scalar.dma_start(out=w_sb[c][:, 0:512],
                                        in_=WvT[c * 128:(c + 1) * 128, 0:512])
                for c in range(NT):
                    nc.gpsimd.dma_start(out=w_sb[c][:, 512:D],
                                        in_=WvT[c * 128:(c + 1) * 128, 512:D])

                def stream_w(w_ext, pool, eng):
                    tiles = []
                    for c in range(NT):
                        t = pool.tile([128, D], MMD, tag="w", name="w_t")
                        eng.dma_start(out=t[:], in_=w_ext[c * 128:(c + 1) * 128, :])
                        tiles.append(t)
                    return tiles

                # parallel hardware DMA queues: Q weights via the scalar
                # queue, K weights via the gpsimd queue
                wq_sb = stream_w(WqT, wqkp, nc.scalar)
                wk_sb = stream_w(WkT, wqkp, nc.gpsimd)

                # ones column FIRST per head: the softmax denominator then
                # lands on PSUM partition 0, where the gpsimd
                # partition_broadcast contract wants its source
                vdsts = []
                for m in range(NT):
                    vdst = Vx[m][:].rearrange("p (h e) -> p h e", e=HD + 1)
                    nc.vector.memset(vdst[:, :, 0:1], 1.0)
                    vdsts.append(vdst)
                with tc.tile_pool(name="psv", bufs=1, space="PSUM") as ppv:
                    psV = {}
                    for n in range(2):
                        for c in range(NT):
                            for m in range(NT):
                                if c == 0:
                                    psV[m] = ppv.tile([128, 512], F32,
                                                      tag=f"pv{m}",
                                                      name=f"pv{m}")
                                nc.tensor.matmul(
                                    psV[m][:],
                                    xT_sb[c][:, m * 128:(m + 1) * 128],
                                    w_sb[c][:, n * 512:(n + 1) * 512],
                                    start=(c == 0), stop=False)
                        for m in range(NT):
                            nc.tensor.matmul(psV[m][:], ones_row[:, :128],
                                             bv_row[:, n * 512:(n + 1) * 512],
                                             start=False, stop=True)
                            nc.vector.tensor_copy(
                                out=vdsts[m][:, n * 8:(n + 1) * 8, 1:HD + 1],
                                in_=psV[m][:].rearrange("p (h e) -> p h e",
                                                        e=HD))
                for m in range(NT):
                    # pad mask: zero whole rows (keys) where ids == PAD,
                    # including the ones column -> denominator excludes them
                    nc.vector.tensor_scalar(
                        out=Vx[m][:], in0=Vx[m][:],
                        scalar1=pad01[:, m:m + 1], scalar2=None,
                        op0=AluOp.mult)

                with tc.tile_pool(name="psp", bufs=4,
                                  space="PSUM") as pp:
                    for m in range(NT):
                        for w_sb2, dst, bias_col in ((wq_sb, Qt, bq_col),
                                                     (wk_sb, Kt, bk_col)):
                            for n in range(2):
                                ps = pp.tile([128, 512], F32, tag="pp",
                                             name="ps")
                                for c in range(NT):
                                    nc.tensor.matmul(
                                        ps[:],
                                        w_sb2[c][:, m * 128:(m + 1) * 128],
                                        xT_sb[c][:, n * 512:(n + 1) * 512],
                                        start=(c == 0), stop=(c == NT - 1))
                                nc.vector.tensor_scalar(
                                    out=dst[m][:, n * 512:(n + 1) * 512],
                                    in0=ps[:],
                                    scalar1=bias_col[:, m:m + 1], scalar2=None,
                                    op0=AluOp.add)

            # ============ Phase C: attention (head pairs) ============
            with tc.tile_pool(name="aot", bufs=1) as ap_pool, \
                 tc.tile_pool(name="wo", bufs=8) as wop:
              aoT = [ap_pool.tile([128, S], MMD, tag=f"ao{t}", name=f"ao{t}")
                     for t in range(NT)]
              with tc.tile_pool(name="expp", bufs=6) as ep, \
                   tc.tile_pool(name="avst", bufs=3) as avs, \
                   tc.tile_pool(name="rrp", bufs=2) as rrp, \
                   tc.tile_pool(name="rbp", bufs=2) as rbp, \
                   tc.tile_pool(name="pssc", bufs=2, space="PSUM") as psc, \
                   tc.tile_pool(name="psav", bufs=1, space="PSUM") as pav:

                # prefetch Wo during attention
                wo_sb = []
                for c in range(NT):
                    w_t = wop.tile([128, D], MMD, tag="wo", name="wo_t")
                    nc.sync.dma_start(out=w_t[:], in_=WoT[c * 128:(c + 1) * 128, :])
                    wo_sb.append(w_t)

                pend = None      # (pair idx, recb tile) awaiting normalization
                for t in range(NT):         # head pair (2t, 2t+1)
                    av_ps = {(par, g): pav.tile([HD + 1, 512], F32,
                                                tag=f"av{par}{g}",
                                                name=f"av{par}{g}")
                             for par in range(2) for g in range(2)}
                    # AV matmuls are emitted one c-iteration behind the
                    # scores, so on the in-order PE queue the next chunk's
                    # score matmuls fill the exp-ACT latency and the PE
                    # never idles (idle resets the 1.2->2.4GHz p-state ramp)
                    def emit_av(c, exs):
                        qs = 128 * c
                        for par in range(2):
                            for g in range(2):
                                if c <= 4 * g + 3:
                                    cs = max(0, qs - 512 * g)
                                    nc.tensor.matmul(
                                        av_ps[(par, g)][:, cs:512],
                                        Vx[c][:, (2 * t + par) * (HD + 1):
                                              (2 * t + par + 1) * (HD + 1)],
                                        exs[par][:, 512 * g + cs:
                                                 512 * (g + 1)],
                                        start=(c == 0),
                                        stop=(c == min(4 * g + 3, NT - 1)))

                    # exp first, then zero the diagonal triangle with a
                    # cheap DVE multiply (keeps the mask off the PE); AV
                    # runs TWO chunks behind so the pair-boundary vector
                    # work can never stall the PE's psum reuse
                    pending = []
                    for c in range(NT):
                        qs = 128 * c        # chunk starts at the diagonal
                        exs = {}
                        for par in range(2):
                            base = par * 64
                            sc = psc.tile([128, 1024], F32, tag="sc", name="sc")
                            if qs < 512:
                                nc.tensor.matmul(
                                    sc[:, qs:512],
                                    Kt[t][base:base + 64, c * 128:(c + 1) * 128],
                                    Qt[t][base:base + 64, qs:512],
                                    start=True, stop=True)
                                nc.tensor.matmul(
                                    sc[:, 512:S],
                                    Kt[t][base:base + 64, c * 128:(c + 1) * 128],
                                    Qt[t][base:base + 64, 512:S],
                                    start=True, stop=True)
                            else:
                                nc.tensor.matmul(
                                    sc[:, qs:S],
                                    Kt[t][base:base + 64, c * 128:(c + 1) * 128],
                                    Qt[t][base:base + 64, qs:S],
                                    start=True, stop=True)
                            ex = ep.tile([128, 1024], MMD, tag="ex",
                                         name="ex")
                            nc.scalar.activation(out=ex[:, qs:S],
                                                 in_=sc[:, qs:S],
                                                 func=Act.Exp, scale=SCALE)
                            nc.vector.tensor_mul(ex[:, qs:qs + 128],
                                                 ex[:, qs:qs + 128],
                                                 masks_sb[:])
                            exs[par] = ex
                        pending.append((c, exs))
                        if len(pending) > 2:
                            emit_av(*pending.pop(0))
                    for item in pending:
                        emit_av(*item)
                    # softmax denominators: fp32 approx-reciprocal (fast
                    # custom DVE op) straight off the PSUM ones-row, fanned
                    # out by gpsimd partition_broadcast, with the aoT
                    # multiply software-pipelined one pair behind — fully
                    # on-chip, nothing ever blocks a DMA queue
                    def finish_pair(pt, precb):
                        for par in range(2):
                            nc.vector.tensor_mul(
                                aoT[pt][par * 64:(par + 1) * 64, :],
                                aoT[pt][par * 64:(par + 1) * 64, :],
                                precb[par][par * 64:(par + 1) * 64, :])

                    rrow = rrp.tile([1, 2 * S], F32, tag="rr", name="rrow")
                    for par in range(2):
                        for g in range(2):
                            nc.vector.reciprocal_approx_fast(
                                out=rrow[0:1, par * S + 512 * g:
                                         par * S + 512 * (g + 1)],
                                in_=av_ps[(par, g)][0:1, :])
                    for par in range(2):
                        st = avs.tile([HD + 1, S], MMD, tag="st", name="st")
                        for g in range(2):
                            nc.vector.tensor_copy(
                                out=st[:, 512 * g:512 * (g + 1)],
                                in_=av_ps[(par, g)][:])
                        nc.sync.dma_start(
                            out=aoT[t][par * 64:par * 64 + HD, :],
                            in_=st[1:HD + 1, :])
                    recb = [rbp.tile([128, S], F32, tag=f"recb{par}",
                                     name=f"recb{par}")
                            for par in range(2)]
                    for par in range(2):
                        nc.gpsimd.partition_broadcast(
                            recb[par][:], rrow[0:1, par * S:(par + 1) * S],
                            channels=128)
                    if pend is not None:
                        finish_pair(*pend)
                    pend = (t, recb)
                finish_pair(*pend)

              # ============ Phase E: output projection ============
              with tc.tile_pool(name="ost", bufs=3) as osp, \
                   tc.tile_pool(name="psf", bufs=4, space="PSUM") as pf:
                for m in range(NT):
                    for n in range(2):
                        ps = pf.tile([128, 512], F32, tag="pf", name="psf")
                        for c in range(NT):
                            nc.tensor.matmul(
                                ps[:],
                                aoT[c][:, m * 128:(m + 1) * 128],
                                wo_sb[c][:, n * 512:(n + 1) * 512],
                                start=(c == 0), stop=False)
                        nc.tensor.matmul(ps[:], ones_row[:, :128],
                                         bo_row[:, n * 512:(n + 1) * 512],
                                         start=False, stop=True)
                        ot = osp.tile([128, 512], F32, tag="ot", name="ot")
                        nc.scalar.copy(out=ot[:], in_=ps[:])
                        nc.sync.dma_start(
                            out=out_e[m * 128:(m + 1) * 128, n * 512:(n + 1) * 512],
                            in_=ot[:])
    nc.finalize()
    return nc


def _np_mm_dtype():
    if MM_MODE == "bf16":
        import ml_dtypes
        return ml_dtypes.bfloat16
    return np.float32


def _host_consts(mmdt):
    jj = np.arange(128)[None, :]
    pp = np.arange(128)[:, None]
    masks = np.where(jj >= pp, 1.0, 0.0).astype(mmdt)
    return masks


def build_in_maps(x, input_ids, Wq, bq, Wk, bk, Wv, bv, Wo, bo):
    x = np.asarray(x, dtype=np.float32)
    input_ids = np.asarray(input_ids)
    mmdt = _np_mm_dtype()
    masks = _host_consts(mmdt)
    bq_r = np.ascontiguousarray(np.asarray(bq, np.float32).reshape(NT, 128).T)
    bk_r = np.ascontiguousarray(np.asarray(bk, np.float32).reshape(NT, 128).T)
    shared = {
        "WqT": np.ascontiguousarray(np.asarray(Wq, np.float32).T).astype(mmdt),
        "WkT": np.ascontiguousarray(np.asarray(Wk, np.float32).T).astype(mmdt),
        "WvT": np.ascontiguousarray(np.asarray(Wv, np.float32).T).astype(mmdt),
        "WoT": np.ascontiguousarray(np.asarray(Wo, np.float32).T).astype(mmdt),
        "bv": np.asarray(bv, np.float32).astype(mmdt),
        "bo": np.asarray(bo, np.float32).astype(mmdt),
        "ones": np.ones([S], mmdt),
        "masks": masks,
    }
    in_maps = []
    for b in range(B):
        ids_r = input_ids[b].astype(np.float32).reshape(NT, 128).T
        m = dict(shared)
        m["xT"] = np.ascontiguousarray(x[b].T).astype(mmdt)
        m["smalls"] = np.ascontiguousarray(
            np.concatenate([ids_r, bq_r, bk_r], axis=1)).astype(np.float32)
        in_maps.append(m)
    return in_maps


def kernel(x, input_ids, Wq, bq, Wk, bk, Wv, bv, Wo, bo):
    global LAST_RESULT, LAST_EXEC_NS
    from concourse.bass_utils import run_bass_kernel_spmd

    if "nc" not in _CACHE:
        _CACHE["nc"] = _build_graph()
    nc = _CACHE["nc"]
    in_maps = build_in_maps(x, input_ids, Wq, bq, Wk, bk, Wv, bv, Wo, bo)

    trace = os.environ.get("KERNEL_TRACE", "0") == "1" and _install_trace_hook()
    res = run_bass_kernel_spmd(nc, in_maps, core_ids=list(range(B)), trace=trace)
    LAST_RESULT = res
    LAST_EXEC_NS = res.exec_time_ns
    return np.stack([res.results[b]["out"] for b in range(B)]).astype(np.float32)



# revision 38
# speedup vs baseline: 1.0561x; 1.0170x over previous
"""Trainium2 Bass kernel for nn_AttentionLayer (B=8, S=1024, D=1024, H=16, HD=64).

Strategy: pure data parallelism — one batch element per NeuronCore (8 cores).
Weights are replicated (pre-transposed on host so the contraction dim lands on
SBUF partitions); x is sharded on batch and pre-transposed per shard.

Per-core compute layout (all transposes eliminated by construction):
  Qt/Kt [dout, s]  = W^T-stacked @ x^T         (d on partitions)
  Vx    [s, dout]  natural, 65-strided per head with a ones column; padded
                    keys' rows are zeroed (this IS the pad mask: they then
                    contribute 0 to both attention output and denominator)
  scoresT[k, q]    = Kt_h^T @ Qt_h             (k on partitions, q free);
                    chunks start at the causal diagonal (qs = 128*c), so
                    only the trapezoid is computed. Causal masking of the
                    128-col diagonal block = identity-matmul accumulating a
                    -1e9 triangle mask into the scores PSUM.
  expT   [k, q]    = exp(scoresT / 8)          (ACT, PSUM->SBUF, bf16 out)
  avT -> out[q, d] via lhsT=[V_h | 1]: ones column also produces the softmax
                    denominator as psum row 64; accumulated per 512-wide
                    q-chunk (4 PSUM banks per pair) with per-key-block
                    column trimming to the causal-active range.
  normalization    fully on-chip: DVE reciprocal reads the denominator row
                    straight from PSUM, GpSimd partition_broadcast fans it
                    out to the 64 head rows, and the aoT *= recip multiply
                    is software-pipelined one head pair behind so the
                    Vector queue never blocks the PE's PSUM reuse.
  out [s, dout]    = attn_outT^T @ Wo^T + bo   (bias via K=1 matmul)

Matmul dtype: bf16 (f32r would be full fp32 precision at the same PE rate,
but hangs TRN2 hardware - observed empirically). End-to-end rel err vs the
fp32 reference is ~4e-3.
"""

import os
import sys
import types

import numpy as np

B, S, D, H, HD = 8, 1024, 1024, 16, 64
NT = D // 128          # 8 partition tiles
PAD_ID = 1.0
NEG = -1e9
SCALE = 1.0 / 8.0      # 1/sqrt(HD)

MM_MODE = os.environ.get("KERNEL_MM_MODE", "bf16")

_CACHE = {}
LAST_RESULT = None
LAST_EXEC_NS = None


def _install_trace_hook():
    """Provide antenv.axon_hooks (missing in this image) so trace=True works."""
    try:
        import antenv
        if "antenv.axon_hooks" in sys.modules:
            return True
        m = types.ModuleType("antenv.axon_hooks")
        _hook = [None]
        m.set_axon_ntff_profile_hook = lambda h: _hook.__setitem__(0, h)
        m.get_axon_ntff_profile_hook = lambda: _hook[0]
        sys.modules["antenv.axon_hooks"] = m
        antenv.axon_hooks = m
        from trn_agent_boot.trn_boot import _ntff_profile_via_ctypes
        hook = _ntff_profile_via_ctypes("/opt/axon/libaxon_pjrt.so")
        if hook is None:
            return False
        m.set_axon_ntff_profile_hook(hook)
        return True
    except Exception:
        return False


def _build_graph():
    import concourse.bass as bass
    import concourse.mybir as mybir
    import concourse.tile as tile
    from concourse import bacc
    from concourse import library_config

    F32 = mybir.dt.float32
    MMD = {"bf16": mybir.dt.bfloat16, "f32r": mybir.dt.float32r,
           "f32": mybir.dt.float32}[MM_MODE]
    AluOp = mybir.AluOpType
    Act = mybir.ActivationFunctionType

    nc = bacc.Bacc(target_bir_lowering=False)

    xT = nc.declare_dram_parameter("xT", [D, S], MMD, isOutput=False)
    WqT = nc.declare_dram_parameter("WqT", [D, D], MMD, isOutput=False)
    WkT = nc.declare_dram_parameter("WkT", [D, D], MMD, isOutput=False)
    WvT = nc.declare_dram_parameter("WvT", [D, D], MMD, isOutput=False)
    WoT = nc.declare_dram_parameter("WoT", [D, D], MMD, isOutput=False)
    bv = nc.declare_dram_parameter("bv", [D], MMD, isOutput=False)
    bo = nc.declare_dram_parameter("bo", [D], MMD, isOutput=False)
    ones_p = nc.declare_dram_parameter("ones", [S], MMD, isOutput=False)
    # smalls: [128, 24] f32 = ids_r | bq_r | bk_r (each [128, 8], host-packed)
    smalls = nc.declare_dram_parameter("smalls", [128, 3 * NT], F32, isOutput=False)
    # 0/1 causal triangle for the 128-wide diagonal block (applied to
    # exp output on the Vector engine, not via PE identity-matmul)
    masks_p = nc.declare_dram_parameter("masks", [128, 128], MMD, isOutput=False)
    out_e = nc.declare_dram_parameter("out", [S, D], F32, isOutput=True)

    with tile.TileContext(nc) as tc:
        # partition_broadcast lives in the `attn` gpsimd library
        nc.gpsimd.load_library(library_config.attn)
        with tc.tile_pool(name="const", bufs=1) as cp, \
             tc.tile_pool(name="qkv", bufs=1) as qp:

            # ---- constants (scalar-triggered DMAs: keep the sync queue
            # free for the x/W streams the first matmuls wait on) ----
            sm = cp.tile([128, 3 * NT], F32, tag="sm", name="sm")
            nc.gpsimd.dma_start(out=sm[:], in_=smalls[:])
            pad01 = cp.tile([128, NT], F32, tag="pad01", name="pad01")
            nc.vector.tensor_scalar(out=pad01[:], in0=sm[:, 0:NT],
                                    scalar1=PAD_ID, scalar2=None,
                                    op0=AluOp.not_equal)
            bq_col = sm[:, NT:2 * NT]
            bk_col = sm[:, 2 * NT:3 * NT]
            bv_row = cp.tile([1, D], MMD, tag="bvr", name="bv_row")
            nc.gpsimd.dma_start(out=bv_row[:], in_=bv[None, :])
            bo_row = cp.tile([1, D], MMD, tag="bor", name="bo_row")
            nc.gpsimd.dma_start(out=bo_row[:], in_=bo[None, :])
            ones_row = cp.tile([1, S], MMD, tag="ones", name="ones_row")
            nc.gpsimd.dma_start(out=ones_row[:], in_=ones_p[None, :])
            masks_sb = cp.tile([128, 128], MMD, tag="masks", name="masks_sb")
            nc.gpsimd.dma_start(out=masks_sb[:], in_=masks_p[:])

            # ---- persistent per-core tensors ----
            Qt = [qp.tile([128, S], MMD, tag=f"qt{t}", name=f"qt{t}")
                  for t in range(NT)]
            Kt = [qp.tile([128, S], MMD, tag=f"kt{t}", name=f"kt{t}")
                  for t in range(NT)]
            Vx = [qp.tile([128, H * (HD + 1)], MMD, tag=f"vx{t}", name=f"vx{t}")
                  for t in range(NT)]

            # ============ Phase B: projections ============
            # V first, then Q/K interleaved per output tile, so attention
            # head-pair t unblocks as soon as Qt[t]/Kt[t] land (instead of
            # after the whole K projection) and the PE never drains across
            # the phase boundary.
            with tc.tile_pool(name="xw", bufs=1) as xp, \
                 tc.tile_pool(name="wst", bufs=8) as wp, \
                 tc.tile_pool(name="wqk", bufs=16) as wqkp:

                # interleave x / Wv column-half DMAs so the c-outermost V
                # projection starts after the first pair lands instead of
                # after the whole 4MB
                xT_sb = [xp.tile([128, S], MMD, tag=f"x{c}", name=f"x{c}")
                         for c in range(NT)]
                w_sb = [wp.tile([128, D], MMD, tag="w", name="w_t")
                        for c in range(NT)]
                for c in range(NT):
                    nc.sync.dma_start(out=xT_sb[c][:], in_=xT[c * 128:(c + 1) * 128, :])
                    nc.scalar.dma_start(out=w_sb[c][:, 0:512],
                                        in_=WvT[c * 128:(c + 1) * 128, 0:512])
                for c in range(NT):
                    nc.gpsimd.dma_start(out=w_sb[c][:, 512:D],
                                        in_=WvT[c * 128:(c + 1) * 128, 512:D])

                def stream_w(w_ext, pool, eng):
                    tiles = []
                    for c in range(NT):
                        t = pool.tile([128, D], MMD, tag="w", name="w_t")
                        eng.dma_start(out=t[:], in_=w_ext[c * 128:(c + 1) * 128, :])
                        tiles.append(t)
                    return tiles

                # parallel hardware DMA queues: Q weights via the scalar
                # queue, K weights via the gpsimd queue
                wq_sb = stream_w(WqT, wqkp, nc.scalar)
                wk_sb = stream_w(WkT, wqkp, nc.gpsimd)

                # ones column FIRST per head: the softmax denominator then
                # lands on PSUM partition 0, where the gpsimd
                # partition_broadcast contract wants its source
                vdsts = []
                for m in range(NT):
                    vdst = Vx[m][:].rearrange("p (h e) -> p h e", e=HD + 1)
                    nc.vector.memset(vdst[:, :, 0:1], 1.0)
                    vdsts.append(vdst)
                with tc.tile_pool(name="psv", bufs=1, space="PSUM") as ppv:
                    psV = {}
                    for n in range(2):
                        for c in range(NT):
                            for m in range(NT):
                                if c == 0:
                                    psV[m] = ppv.tile([128, 512], F32,
                                                      tag=f"pv{m}",
                                                      name=f"pv{m}")
                                nc.tensor.matmul(
                                    psV[m][:],
                                    xT_sb[c][:, m * 128:(m + 1) * 128],
                                    w_sb[c][:, n * 512:(n + 1) * 512],
                                    start=(c == 0), stop=False)
                        for m in range(NT):
                            nc.tensor.matmul(psV[m][:], ones_row[:, :128],
                                             bv_row[:, n * 512:(n + 1) * 512],
                                             start=False, stop=True)
                            nc.vector.tensor_copy(
                                out=vdsts[m][:, n * 8:(n + 1) * 8, 1:HD + 1],
                                in_=psV[m][:].rearrange("p (h e) -> p h e",
                                                        e=HD))
                for m in range(NT):
                    # pad mask: zero whole rows (keys) where ids == PAD,
                    # including the ones column -> denominator excludes them
                    nc.vector.tensor_scalar(
                        out=Vx[m][:], in0=Vx[m][:],
                        scalar1=pad01[:, m:m + 1], scalar2=None,
                        op0=AluOp.mult)

                with tc.tile_pool(name="psp", bufs=4,
                                  space="PSUM") as pp:
                    for m in range(NT):
                        for w_sb2, dst, bias_col in ((wq_sb, Qt, bq_col),
                                                     (wk_sb, Kt, bk_col)):
                            for n in range(2):
                                ps = pp.tile([128, 512], F32, tag="pp",
                                             name="ps")
                                for c in range(NT):
                                    nc.tensor.matmul(
                                        ps[:],
                                        w_sb2[c][:, m * 128:(m + 1) * 128],
                                        xT_sb[c][:, n * 512:(n + 1) * 512],
                                        start=(c == 0), stop=(c == NT - 1))
                                nc.vector.tensor_scalar(
                                    out=dst[m][:, n * 512:(n + 1) * 512],
                                    in0=ps[:],
                                    scalar1=bias_col[:, m:m + 1], scalar2=None,
                                    op0=AluOp.add)

            # ============ Phase C: attention (head pairs) ============
            with tc.tile_pool(name="aot", bufs=1) as ap_pool, \
                 tc.tile_pool(name="wo", bufs=8) as wop:
              aoT = [ap_pool.tile([128, S], MMD, tag=f"ao{t}", name=f"ao{t}")
                     for t in range(NT)]
              with tc.tile_pool(name="expp", bufs=6) as ep, \
                   tc.tile_pool(name="avst", bufs=3) as avs, \
                   tc.tile_pool(name="rrp", bufs=2) as rrp, \
                   tc.tile_pool(name="rbp", bufs=2) as rbp, \
                   tc.tile_pool(name="pssc", bufs=2, space="PSUM") as psc, \
                   tc.tile_pool(name="psav", bufs=1, space="PSUM") as pav:

                # prefetch Wo during attention
                wo_sb = []
                for c in range(NT):
                    w_t = wop.tile([128, D], MMD, tag="wo", name="wo_t")
                    nc.sync.dma_start(out=w_t[:], in_=WoT[c * 128:(c + 1) * 128, :])
                    wo_sb.append(w_t)

                pend = None      # (pair idx, recb tile) awaiting normalization
                for t in range(NT):         # head pair (2t, 2t+1)
                    av_ps = {(par, g): pav.tile([HD + 1, 512], F32,
                                                tag=f"av{par}{g}",
                                                name=f"av{par}{g}")
                             for par in range(2) for g in range(2)}
                    # AV matmuls are emitted one c-iteration behind the
                    # scores, so on the in-order PE queue the next chunk's
                    # score matmuls fill the exp-ACT latency and the PE
                    # never idles (idle resets the 1.2->2.4GHz p-state ramp)
                    def emit_av(c, exs):
                        qs = 128 * c
                        for par in range(2):
                            for g in range(2):
                                if c <= 4 * g + 3:
                                    cs = max(0, qs - 512 * g)
                                    nc.tensor.matmul(
                                        av_ps[(par, g)][:, cs:512],
                                        Vx[c][:, (2 * t + par) * (HD + 1):
                                              (2 * t + par + 1) * (HD + 1)],
                                        exs[par][:, 512 * g + cs:
                                                 512 * (g + 1)],
                                        start=(c == 0),
                                        stop=(c == min(4 * g + 3, NT - 1)))

                    # exp first, then zero the diagonal triangle with a
                    # cheap DVE multiply (keeps the mask off the PE); AV
                    # runs TWO chunks behind so the pair-boundary vector
                    # work can never stall the PE's psum reuse
                    pending = []
                    for c in range(NT):
                        qs = 128 * c        # chunk starts at the diagonal
                        exs = {}
                        for par in range(2):
                            base = par * 64
                            sc = psc.tile([128, 1024], F32, tag="sc", name="sc")
                            if qs < 512:
                                nc.tensor.matmul(
                                    sc[:, qs:512],
                                    Kt[t][base:base + 64, c * 128:(c + 1) * 128],
                                    Qt[t][base:base + 64, qs:512],
                                    start=True, stop=True)
                                nc.tensor.matmul(
                                    sc[:, 512:S],
                                    Kt[t][base:base + 64, c * 128:(c + 1) * 128],
                                    Qt[t][base:base + 64, 512:S],
                                    start=True, stop=True)
                            else:
                                nc.tensor.matmul(
                                    sc[:, qs:S],
                                    Kt[t][base:base + 64, c * 128:(c + 1) * 128],
                                    Qt[t][base:base + 64, qs:S],
                                    start=True, stop=True)
                            ex = ep.tile([128, 1024], MMD, tag="ex",
                                         name="ex")
                            nc.scalar.activation(out=ex[:, qs:S],
                                                 in_=sc[:, qs:S],
                                                 func=Act.Exp, scale=SCALE)
                            nc.vector.tensor_mul(ex[:, qs:qs + 128],
                                                 ex[:, qs:qs + 128],
                                                 masks_sb[:])
                            exs[par] = ex
                        pending.append((c, exs))
                        if len(pending) > 2:
                            emit_av(*pending.pop(0))
                    for item in pending:
                        emit_av(*item)
                    # softmax denominators: fp32 approx-reciprocal (fast
                    # custom DVE op) straight off the PSUM ones-row, fanned
                    # out by gpsimd partition_broadcast, with the aoT
                    # multiply software-pipelined one pair behind — fully
                    # on-chip, nothing ever blocks a DMA queue
                    def finish_pair(pt, precb):
                        for par in range(2):
                            nc.vector.tensor_mul(
                                aoT[pt][par * 64:(par + 1) * 64, :],
                                aoT[pt][par * 64:(par + 1) * 64, :],
                                precb[par][par * 64:(par + 1) * 64, :])

                    rrow = rrp.tile([1, 2 * S], F32, tag="rr", name="rrow")
                    for par in range(2):
                        for g in range(2):
                            nc.vector.reciprocal_approx_fast(
                                out=rrow[0:1, par * S + 512 * g:
                                         par * S + 512 * (g + 1)],
                                in_=av_ps[(par, g)][0:1, :])
                    for par in range(2):
                        st = avs.tile([HD + 1, S], MMD, tag="st", name="st")
                        for g in range(2):
                            nc.vector.tensor_copy(
                                out=st[:, 512 * g:512 * (g + 1)],
                                in_=av_ps[(par, g)][:])
                        nc.sync.dma_start(
                            out=aoT[t][par * 64:par * 64 + HD, :],
                            in_=st[1:HD + 1, :])
                    recb = [rbp.tile([128, S], F32, tag=f"recb{par}",
                                     name=f"recb{par}")
                            for par in range(2)]
                    for par in range(2):
                        nc.gpsimd.partition_broadcast(
                            recb[par][:], rrow[0:1, par * S:(par + 1) * S],
                            channels=128)
                    if pend is not None:
                        finish_pair(*pend)
                    pend = (t, recb)
                finish_pair(*pend)

              # ============ Phase E: output projection ============
              with tc.tile_pool(name="ost", bufs=3) as osp, \
                   tc.tile_pool(name="psf", bufs=4, space="PSUM") as pf:
                for m in range(NT):
                    for n in range(2):
                        ps = pf.tile([128, 512], F32, tag="pf", name="psf")
                        for c in range(NT):
                            nc.tensor.matmul(
                                ps[:],
                                aoT[c][:, m * 128:(m + 1) * 128],
                                wo_sb[c][:, n * 512:(n + 1) * 512],
                                start=(c == 0), stop=False)
                        nc.tensor.matmul(ps[:], ones_row[:, :128],
                                         bo_row[:, n * 512:(n + 1) * 512],
                                         start=False, stop=True)
                        ot = osp.tile([128, 512], F32, tag="ot", name="ot")
                        nc.scalar.copy(out=ot[:], in_=ps[:])
                        nc.sync.dma_start(
                            out=out_e[m * 128:(m + 1) * 128, n * 512:(n + 1) * 512],
                            in_=ot[:])
    nc.finalize()
    return nc


def _np_mm_dtype():
    if MM_MODE == "bf16":
        import ml_dtypes
        return ml_dtypes.bfloat16
    return np.float32


def _host_consts(mmdt):
    jj = np.arange(128)[None, :]
    pp = np.arange(128)[:, None]
    masks = np.where(jj >= pp, 1.0, 0.0).astype(mmdt)
    return masks


def build_in_maps(x, input_ids, Wq, bq, Wk, bk, Wv, bv, Wo, bo):
    x = np.asarray(x, dtype=np.float32)
    input_ids = np.asarray(input_ids)
    mmdt = _np_mm_dtype()
    masks = _host_consts(mmdt)
    bq_r = np.ascontiguousarray(np.asarray(bq, np.float32).reshape(NT, 128).T)
    bk_r = np.ascontiguousarray(np.asarray(bk, np.float32).reshape(NT, 128).T)
    shared = {
        "WqT": np.ascontiguousarray(np.asarray(Wq, np.float32).T).astype(mmdt),
        "WkT": np.ascontiguousarray(np.asarray(Wk, np.float32).T).astype(mmdt),
        "WvT": np.ascontiguousarray(np.asarray(Wv, np.float32).T).astype(mmdt),
        "WoT": np.ascontiguousarray(np.asarray(Wo, np.float32).T).astype(mmdt),
        "bv": np.asarray(bv, np.float32).astype(mmdt),
        "bo": np.asarray(bo, np.float32).astype(mmdt),
        "ones": np.ones([S], mmdt),
        "masks": masks,
    }
    in_maps = []
    for b in range(B):
        ids_r = input_ids[b].astype(np.float32).reshape(NT, 128).T
        m = dict(shared)
        m["xT"] = np.ascontiguousarray(x[b].T).astype(mmdt)
        m["smalls"] = np.ascontiguousarray(
            np.concatenate([ids_r, bq_r, bk_r], axis=1)).astype(np.float32)
        in_maps.append(m)
    return in_maps


def kernel(x, input_ids, Wq, bq, Wk, bk, Wv, bv, Wo, bo):
    global LAST_RESULT, LAST_EXEC_NS
    from concourse.bass_utils import run_bass_kernel_spmd

    if "nc" not in _CACHE:
        _CACHE["nc"] = _build_graph()
    nc = _CACHE["nc"]
    in_maps = build_in_maps(x, input_ids, Wq, bq, Wk, bk, Wv, bv, Wo, bo)

    trace = os.environ.get("KERNEL_TRACE", "0") == "1" and _install_trace_hook()
    res = run_bass_kernel_spmd(nc, in_maps, core_ids=list(range(B)), trace=trace)
    LAST_RESULT = res
    LAST_EXEC_NS = res.exec_time_ns
    return np.stack([res.results[b]["out"] for b in range(B)]).astype(np.float32)

